# revision 1
# baseline (speedup 1.0000x reference)
"""Fused multi-head self-attention kernel for Trainium2 (Bass/Tile).

Problem: x:[4,2560,320] f32, Wq/Wk/Wv:[320,512], Wo:[512,320], bo:[320]
  q,k,v = x@W*, 8 heads x 64; sim = q k^T * d^-0.5; attn = softmax(sim);
  out = (attn @ v) @ Wo + bo.

Sharding: batch*head 32-way -> 8 cores: core c handles batch c//2 and the
4-head group c%2. Host sums the two half-head partial output projections
per batch and adds the bias.

Per-core layout trick: scores are computed TRANSPOSED (sT[j,i] = k_j . q_i)
so that the softmax denominator arrives for free: v gets a ones-column
appended, and out' = expT_slice.T @ [v|1] accumulates both attn@v and the
row sums. Normalisation is then a per-partition scalar multiply.
"""

import sys

import numpy as np

if "/opt/trn_rl_repo" not in sys.path:
    sys.path.insert(0, "/opt/trn_rl_repo")

from contextlib import ExitStack

import concourse.bass as bass
from concourse import bacc
import concourse.mybir as mybir
import concourse.tile as tile
from concourse.bass_utils import run_bass_kernel_spmd
from concourse.masks import make_identity

# ---- problem constants (hardcoded per contract) ----
B = 4
N = 2560
QD = 320
H_TOT = 8
D = 64
HPC = 4                  # heads per core
IPC = HPC * D            # 256 inner dims per core
SCALE = D ** -0.5
NT = N // 128            # 20 n-tiles
HALF = N // 2            # 1280
F32 = mybir.dt.float32
BF16 = mybir.dt.bfloat16
F32R = mybir.dt.float32r
EXP = mybir.ActivationFunctionType.Exp

# qd (=320) split into K subtiles for the 128-partition contraction
KS = [(0, 128), (128, 128), (256, 64)]
# 1280-wide column chunks (PSUM-bank-aligned matmul N<=512)
CHUNKS = [(0, 512), (512, 512), (1024, 256)]

EXP_BUFS = 36            # expT pool slots of [128,1280] bf16 (2.5KB/part each)

_built = {}
last_results = None      # stashed BassKernelResults for the test harness


def _build():
    nc = bacc.Bacc(None, target_bir_lowering=False)
    xT = nc.declare_dram_parameter("xT", [QD, N], F32R, isOutput=False)
    wq = nc.declare_dram_parameter("wq", [QD, IPC], F32R, isOutput=False)
    wk = nc.declare_dram_parameter("wk", [QD, IPC], F32R, isOutput=False)
    wv = nc.declare_dram_parameter("wv", [QD, IPC], F32R, isOutput=False)
    wo = nc.declare_dram_parameter("wo", [IPC, QD], F32R, isOutput=False)
    y = nc.declare_dram_parameter("y", [N, QD], F32, isOutput=True)

    with tile.TileContext(nc) as tc, ExitStack() as ctx:
        const = ctx.enter_context(tc.tile_pool(name="const", bufs=1))
        smps = ctx.enter_context(tc.tile_pool(name="smps", bufs=2, space="PSUM"))
        epool = ctx.enter_context(tc.tile_pool(name="epool", bufs=EXP_BUFS))
        sbsm = ctx.enter_context(tc.tile_pool(name="sbsm", bufs=4))
        ypool = ctx.enter_context(tc.tile_pool(name="ypool", bufs=3))
        spool_cm = tc.tile_pool(name="spool", bufs=2, space="PSUM")
        spool = spool_cm.__enter__()

        ident = const.tile([128, 128], F32, tag="ident", name="ident")
        make_identity(nc, ident[:])
        warm = sbsm.tile([128, 1], F32, tag="warm", name="warm")
        nc.scalar.activation(warm[:], ident[:, 0:1], EXP, scale=1.0)
        for _ in range(6):
            pw = smps.tile([128, 128], F32, tag="sm", name="pwarm")
            nc.tensor.matmul(pw[:], lhsT=ident[:], rhs=ident[:],
                             start=True, stop=True)

        # ---- persistent inputs (DMA emission ordered by first use) ----
        xts = [const.tile([128, N], F32R, tag=f"xt{ki}", name=f"xt{ki}")
               for ki in range(3)]
        wqs = [const.tile([128, IPC], F32R, tag=f"wq{ki}", name=f"wq{ki}")
               for ki in range(3)]
        wks = [const.tile([128, IPC], F32R, tag=f"wk{ki}", name=f"wk{ki}")
               for ki in range(3)]
        wvs = [const.tile([128, IPC], F32R, tag=f"wv{ki}", name=f"wv{ki}")
               for ki in range(3)]
        wos = [const.tile([128, QD], F32R, tag=f"wo{kk}", name=f"wo{kk}")
               for kk in range(2)]
        # critical set first; x chunks on sync queue, weights on gpsimd queue
        for ki, (k0, kw) in enumerate(KS):
            nc.sync.dma_start(xts[ki][:kw, 0:640], xT[k0:k0 + kw, 0:640])
            nc.gpsimd.dma_start(wqs[ki][:kw, :], wq[k0:k0 + kw, :])
        for ki, (k0, kw) in enumerate(KS):
            nc.sync.dma_start(xts[ki][:kw, 640:1280], xT[k0:k0 + kw, 640:1280])
            nc.gpsimd.dma_start(wks[ki][:kw, :], wk[k0:k0 + kw, :])
        for cc in range(2, 4):
            for ki, (k0, kw) in enumerate(KS):
                nc.sync.dma_start(xts[ki][:kw, cc * 640:(cc + 1) * 640],
                                  xT[k0:k0 + kw, cc * 640:(cc + 1) * 640])
        for ki, (k0, kw) in enumerate(KS):
            nc.gpsimd.dma_start(wvs[ki][:kw, :], wv[k0:k0 + kw, :])
        for kk in range(2):
            nc.gpsimd.dma_start(wos[kk][:], wo[kk * 128:(kk + 1) * 128, :])

        # qT/kT: [inner(256) x n] as 2 tiles of [128, N] each; fp32 storage
        qk_sb = [const.tile([128, N], F32R, tag=f"qk{i}", name=f"qk{i}") for i in range(4)]
        # outT: normalized attention output, [inner x n]
        outT = [const.tile([128, N], F32R, tag=f"oT{kk}", name=f"oT{kk}") for kk in range(2)]
        # v with ones column per head: [n-tile][128, 4*65] bf16
        v1s = [const.tile([128, HPC * 65], BF16, tag=f"v1_{j}", name=f"v1_{j}") for j in range(NT)]

        ws = [wqs, wks]
        tails = {}

        def qk_proj(ti, m, half, chunks=None):
            """qT/kT tile ti(0=q,1=k), inner slab m, col half -> qk_sb[ti*2+m]."""
            for c0, cw in (chunks or CHUNKS):
                ps = smps.tile([128, 512], F32, tag="sm", name="smp")
                for ki, (k0, kw) in enumerate(KS):
                    nc.tensor.matmul(
                        ps[:, 0:cw],
                        lhsT=ws[ti][ki][:kw, m * 128:(m + 1) * 128],
                        rhs=xts[ki][:kw, half * HALF + c0:half * HALF + c0 + cw],
                        start=(ki == 0), stop=(ki == 2),
                    )
                nc.vector.tensor_copy(
                    qk_sb[ti * 2 + m][:, half * HALF + c0:half * HALF + c0 + cw],
                    ps[:, 0:cw])

        def v_proj(j):
            """v for n-tile j (all 4 heads) -> v1s[j] bf16 with ones cols."""
            ps = smps.tile([128, IPC], F32, tag="sm", name="smv")
            for ki, (k0, kw) in enumerate(KS):
                nc.tensor.matmul(
                    ps[:],
                    lhsT=xts[ki][:kw, j * 128:(j + 1) * 128],
                    rhs=wvs[ki][:kw, :],
                    start=(ki == 0), stop=(ki == 2),
                )
            v1v = v1s[j][:].rearrange("p (h e) -> p h e", e=65)
            nc.gpsimd.memset(v1v[:, :, 64:65], 1.0)
            nc.vector.tensor_copy(
                v1v[:, :, 0:64], ps[:].rearrange("p (h d) -> p h d", d=64))

        def scores_exp(h, half, j):
            """sT[j-tile, i-half] = k_j . q_i (f32r), then exp -> bf16 SBUF."""
            m, po = h // 2, (h % 2) * 64
            ps = spool.tile([128, HALF], F32, tag="s", name="s")
            for c0, cw in CHUNKS:
                nc.tensor.matmul(
                    ps[:, c0:c0 + cw],
                    lhsT=qk_sb[2 + m][po:po + 64, j * 128:(j + 1) * 128],
                    rhs=qk_sb[m][po:po + 64, half * HALF + c0:half * HALF + c0 + cw],
                    start=True, stop=True,
                )
            et = epool.tile([128, HALF], BF16, tag="e", name="et")
            nc.scalar.activation(et[:], ps[:], EXP, scale=float(SCALE))
            return et

        def attn_step(h, half, ets, i, tail=False):
            """out'[i-tile] = sum_j expT_j[:, i].T @ [v|1]; normalize; transpose."""
            m, po = h // 2, (h % 2) * 64
            pso = (tails["pool"].tile([128, 65], F32, tag="to", name="smo")
                   if tail else smps.tile([128, 65], F32, tag="sm", name="smo"))
            for j in range(NT):
                nc.tensor.matmul(
                    pso[:],
                    lhsT=ets[j][:, i * 128:(i + 1) * 128],
                    rhs=v1s[j][:, h * 65:(h + 1) * 65],
                    start=(j == 0), stop=(j == NT - 1),
                )
            rc = sbsm.tile([128, 1], F32, tag="rc", name="rc")
            nc.vector.reciprocal(rc[:], pso[:, 64:65])
            on = sbsm.tile([128, 64], F32, tag="on", name="on")
            nc.vector.tensor_scalar_mul(on[:], pso[:, 0:64], rc[:])
            pst = (tails["pool"].tile([64, 128], F32, tag="tt", name="smt")
                   if tail else smps.tile([64, 128], F32, tag="sm", name="smt"))
            nc.tensor.transpose(pst[:], on[:], ident[:])
            ig = half * 10 + i
            nc.vector.tensor_copy(outT[m][po:po + 64, ig * 128:(ig + 1) * 128], pst[:])

        def y_step(i, tail=False):
            """y[i-tile] = outT[:, i].T @ Wo -> DRAM."""
            psy = (tails["pool"].tile([128, QD], F32, tag="ty", name="smy")
                   if tail else smps.tile([128, QD], F32, tag="sm", name="smy"))
            for kk in range(2):
                nc.tensor.matmul(
                    psy[:],
                    lhsT=outT[kk][:, i * 128:(i + 1) * 128],
                    rhs=wos[kk][:],
                    start=(kk == 0), stop=(kk == 1),
                )
            ysb = ypool.tile([128, QD], F32, tag="y", name="ysb")
            nc.vector.tensor_copy(ysb[:], psy[:])
            nc.sync.dma_start(y[i * 128:i * 128 + 64, :], ysb[0:64, :])
            nc.gpsimd.dma_start(y[i * 128 + 64:(i + 1) * 128, :], ysb[64:128, :])

        # ---- emission: minimal upfront proj, rest interleaved ----
        qk_proj(0, 0, 0)
        qk_proj(1, 0, 0, chunks=CHUNKS[:1])
        qk_proj(1, 0, 0, chunks=CHUNKS[1:])
        # (ti, m, half) projections still pending, keyed by (h, half, j) slot
        pending = {(0, 0, 0): (1, 0, 1), (0, 0, 2): (0, 0, 1),
                   (0, 1, 11): (0, 1, 0), (0, 1, 14): (1, 1, 0),
                   (1, 0, 11): (1, 1, 1), (1, 0, 14): (0, 1, 1)}

        prev = None
        for h in range(HPC):
            for half in range(2):
                ets = []
                for j in range(NT):
                    ets.append(scores_exp(h, half, j))
                    pr = pending.pop((h, half, j), None)
                    if pr is not None:
                        qk_proj(*pr)
                    if h == 0 and half == 0:
                        v_proj(j)
                    elif prev is not None and j < 10:
                        ph, phalf, pets = prev
                        attn_step(ph, phalf, pets, j)
                        if ph == 3 and phalf == 0:
                            y_step(j)
                prev = (h, half, ets)
        spool_cm.__exit__(None, None, None)
        tpool = ctx.enter_context(tc.tile_pool(name="tpool", bufs=2, space="PSUM"))
        tails["pool"] = tpool
        for i in range(10):
            attn_step(3, 1, prev[2], i, tail=True)
            y_step(10 + i, tail=True)

    nc.compile()
    return nc


def _get_nc():
    if "nc" not in _built:
        _built["nc"] = _build()
    return _built["nc"]


def kernel(x, Wq, Wk, Wv, Wo, bo):
    global last_results
    x = np.asarray(x, dtype=np.float32)
    Wq = np.asarray(Wq, dtype=np.float32)
    Wk = np.asarray(Wk, dtype=np.float32)
    Wv = np.asarray(Wv, dtype=np.float32)
    Wo = np.asarray(Wo, dtype=np.float32)
    bo = np.asarray(bo, dtype=np.float32)

    nc = _get_nc()
    in_maps = []
    for c in range(8):
        bb, g = divmod(c, 2)
        sl = slice(g * IPC, (g + 1) * IPC)
        in_maps.append({
            "xT": np.ascontiguousarray(x[bb].T),
            "wq": np.ascontiguousarray(Wq[:, sl]),
            "wk": np.ascontiguousarray(Wk[:, sl]),
            "wv": np.ascontiguousarray(Wv[:, sl]),
            "wo": np.ascontiguousarray(Wo[sl, :]),
        })
    res = run_bass_kernel_spmd(nc, in_maps, core_ids=list(range(8)))
    last_results = res
    parts = [r["y"] for r in res.results]
    out = np.empty((B, N, QD), dtype=np.float32)
    for bb in range(B):
        out[bb] = parts[2 * bb] + parts[2 * bb + 1]
    out += bo
    return out

